# revision 42
# baseline (speedup 1.0000x reference)
"""Policy-masked multi-head attention for Trainium2 (Bass/Tile).

Full-input contract: kernel(**inputs) takes the complete tensors and returns
the complete (N, B, C) output. Internally data-parallel over the batch dim:
core b computes batch b on one NeuronCore (no collectives). Host-side prep
is layout only: per-core slicing and an x transpose so the device streams
activations in the (c, n) orientation the PE wants.

Per-core math (N=1024 tokens, C=768, H=12 heads, hd=64):
  qkv = x @ W_qkv + b_qkv ; per head: S = q k^T / 8
  A   = exp(S) * mask      (mask = p[key] off-diag, 1 on diag)
  out = (A @ v) / (sum_k A + eps) ; y = out @ W_proj + b_proj
Softmax max-subtraction is skipped (exp args bounded, |s/8| <~ 8): identical
result modulo the ~1e-6-relative eps regularizers.

Dataflow (everything PE-friendly, no on-chip transposes):
  xT (c,n) loaded directly
  qT,kT per head-pair: psum[j,n] = W_qkv[c,j]^T @ xT[c,n]; the pair's two
    heads live in the 64-row halves of the 128-row j-tile
  v in natural (n,d) orientation with appended ones lanes
  S_T[nk,nq] = kT^T qT, both heads packed in the PE array (row groups 0/64)
  policy folded into exp as a per-partition (key-indexed) log bias; the
  always-keep diagonal is restored by adding -8*ln(p) to the 128-wide
  diagonal block of the psum before the exp
  A_T[nk,nq] (bf16) -> AV with v stationary; the ones lanes make the PE
  emit softmax denominators for free (head A psum row 64, head B row 32)
  out_T (c,nq) feeds the projection as lhsT, producing natural (n,c) psum
  tiles that DMA straight to HBM.
"""

import sys

if "/opt/trn_rl_repo" not in sys.path:
    sys.path.insert(0, "/opt/trn_rl_repo")

import numpy as np

N, B, C = 1024, 8, 768
NH = 12          # heads
HD = 64          # head dim
P = 128          # partitions
NT = N // P      # 8 token tiles
CT = C // P      # 6 channel tiles
HP = NH // 2     # 6 head pairs
SCALE = 0.125    # hd**-0.5
EPS = 1e-6
# per (tile, pair) v-lane layout: [vA(64) | 1 | 0(31) | 1 | 0(31) | vB(64)]
# head A stationary window = cols [0:65)   -> psum rows 0:64 out, row 64 denom
# head B stationary window = cols [64:192) -> psum rows 64:128 out, row 32 denom
#   (window col 0 hits head A's ones lane -> junk in psum row 0, unused;
#    denominator rows must sit at 32-aligned partitions)
VW = 192

_CACHE = {}


def _build():
    if "nc" in _CACHE:
        return _CACHE["nc"]

    from contextlib import ExitStack

    import concourse.bass as bass
    import concourse.tile as tile
    from concourse import bacc, mybir
    from concourse.masks import make_identity

    f32 = mybir.dt.float32
    f32r = mybir.dt.float32r
    bf16 = mybir.dt.bfloat16
    Alu = mybir.AluOpType
    Act = mybir.ActivationFunctionType

    nc = bacc.Bacc()

    xT_d = nc.declare_dram_parameter("xT", [C, N], f32r, isOutput=False)
    pol_d = nc.declare_dram_parameter("policy", [N], f32, isOutput=False)
    wqkv_d = nc.declare_dram_parameter("W_qkv", [C, 3 * C], f32r, isOutput=False)
    bqkv_d = nc.declare_dram_parameter("b_qkv", [3 * C], f32, isOutput=False)
    wproj_d = nc.declare_dram_parameter("W_proj", [C, C], f32r, isOutput=False)
    bproj_d = nc.declare_dram_parameter("b_proj", [C], f32, isOutput=False)
    y_d = nc.declare_dram_parameter("y", [N, C], f32, isOutput=True)

    xT_v = xT_d.rearrange("(ct p) n -> p ct n", p=P)      # (128, 6, 1024)
    pol_v = pol_d.rearrange("(t p) -> p t", p=P)          # (128, 8)
    wqkv_v = wqkv_d.rearrange("(ct p) j -> p ct j", p=P)  # (128, 6, 2304)
    wproj_v = wproj_d.rearrange("(ct p) j -> p ct j", p=P)
    bqk_v = bqkv_d[0 : 2 * C].rearrange("(t p) -> p t", p=P)  # (128, 12)
    y_v = y_d.rearrange("(t p) c -> p t c", p=P)

    def bcast(dram_ap, parts):
        # partition-broadcast read of a 1-D dram slice (step-0 partition dim)
        return bass.AP(
            tensor=dram_ap.tensor, offset=dram_ap.offset, ap=[[0, parts]] + dram_ap.ap
        )

    with tile.TileContext(nc) as tc, ExitStack() as ctx:
        # ---- pools -------------------------------------------------------
        persist = ctx.enter_context(tc.tile_pool(name="persist", bufs=1))
        qkT_pool = ctx.enter_context(tc.tile_pool(name="qkT", bufs=3))
        A_pool = ctx.enter_context(tc.tile_pool(name="Apool", bufs=2))
        wvstack = ExitStack()
        wvp = wvstack.enter_context(tc.tile_pool(name="wv", bufs=1))

        # ---- input loads, in consumption order --------------------------
        # phase 2 (v proj) needs xT[ct] + wv[ct]: interleave those DMAs first
        xT = persist.tile([P, CT, N], f32r)       # 24 KB/part
        wv_sb = wvp.tile([P, CT, C], f32r)        # 18 KB/part, scoped
        for ct in range(CT):
            nc.sync.dma_start(xT[:, ct, :], xT_v[:, ct, :])
            nc.sync.dma_start(wv_sb[:, ct, :], wqkv_v[:, ct, 2 * C : 3 * C])

        ident = persist.tile([P, P], f32)  # diagonal mask for the keep-fix
        make_identity(nc, ident)

        pol_sb = persist.tile([P, NT], f32)
        nc.sync.dma_start(pol_sb, pol_v)
        logp = persist.tile([P, NT], f32)
        # clamp away exact zeros so ln() stays finite (ln(1e-38) = -87.5)
        nc.vector.tensor_scalar_max(logp, pol_sb, 1e-38)
        nc.scalar.activation(logp, logp, Act.Ln)
        n8logp = persist.tile([P, NT], f32)
        nc.vector.tensor_scalar_mul(n8logp, logp, -8.0)

        bqk_sb = persist.tile([P, 2 * CT], f32)
        nc.sync.dma_start(bqk_sb, bqk_v)
        bv_bc = persist.tile([P, C], f32)
        nc.sync.dma_start(bv_bc, bcast(bqkv_d[2 * C : 3 * C], P))
        bproj_bc = persist.tile([P, C], f32)
        nc.sync.dma_start(bproj_bc, bcast(bproj_d[:], P))

        wqk_sb = persist.tile([P, CT, 2 * C], f32r)   # 36 KB/part
        for ct in range(CT):
            nc.sync.dma_start(wqk_sb[:, ct, :], wqkv_v[:, ct, 0 : 2 * C])
        wproj_sb = persist.tile([P, CT, C], f32r)     # 18 KB/part
        for ct in range(CT):
            nc.sync.dma_start(wproj_sb[:, ct, :], wproj_v[:, ct, :])

        vv = persist.tile([P, NT, HP, VW], bf16)  # v + ones lanes, 18 KB/part
        outT = persist.tile([P, HP, N], f32r)      # attention out, transposed

        # K=1 stationary ones columns used to broadcast the denominator rows
        # across partitions via the PE (outer product). Head A's denom sits at
        # dsb row 64 -> broadcast to psum rows 0:64; head B's at dsb row 0 ->
        # rows 64:128. (Two different-base matmuls must NOT share one psum
        # accumulation group -- that faults the device -- so two tiles.)
        onesA_f = persist.tile([HD + 1, HD], f32)
        nc.vector.memset(onesA_f, 1.0)
        onesA = persist.tile([HD + 1, HD], f32r)
        nc.vector.tensor_copy(onesA, onesA_f)
        onesB_f = persist.tile([1, P], f32)
        nc.vector.memset(onesB_f[:, 0:HD], 0.0)
        nc.vector.memset(onesB_f[:, HD:P], 1.0)
        onesB = persist.tile([1, P], f32r)
        nc.vector.tensor_copy(onesB, onesB_f)

        # ones/zeros lanes of the v layout
        nc.gpsimd.memset(vv[:, :, :, HD : 2 * HD], 0.0)
        nc.gpsimd.memset(vv[:, :, :, HD : HD + 1], 1.0)

        # ---- phase 2: v projection (natural orientation) ----------------
        with tc.tile_pool(name="psv", bufs=2, space="PSUM") as psv:
            for t in range(NT):
                pv = psv.tile([P, C], f32, tag="pv")
                for ct in range(CT):
                    lhs = xT[:, ct, t * P : (t + 1) * P]
                    nc.tensor.matmul(
                        pv[:, 0:512], lhs, wv_sb[:, ct, 0:512],
                        start=(ct == 0), stop=(ct == CT - 1),
                    )
                    nc.tensor.matmul(
                        pv[:, 512:768], lhs, wv_sb[:, ct, 512:768],
                        start=(ct == 0), stop=(ct == CT - 1),
                    )
                # scatter halves into the padded v layout, add bias, cast bf16
                pv3 = pv.rearrange("p (hp w) -> p hp w", w=P)
                bv3 = bv_bc.rearrange("p (hp w) -> p hp w", w=P)
                nc.vector.tensor_add(vv[:, t, :, 0:HD], pv3[:, :, 0:HD], bv3[:, :, 0:HD])
                nc.vector.tensor_add(
                    vv[:, t, :, P : P + HD], pv3[:, :, HD:P], bv3[:, :, HD:P]
                )
        wvstack.close()  # wv consumed; free its SBUF

        # ---- phase 3: per head-pair: qT,kT -> S_T -> exp -> AV ----------
        with tc.tile_pool(name="psqk", bufs=1, space="PSUM") as psqk, tc.tile_pool(
            name="pss", bufs=2, space="PSUM"
        ) as pss, tc.tile_pool(name="psav", bufs=1, space="PSUM") as psav, tc.tile_pool(
            name="dsbp", bufs=2
        ) as dsbp, tc.tile_pool(name="rbp", bufs=2) as rbp:
            for hp in range(HP):
                # --- qT, kT for this pair (two 128-row j-tiles) ---
                pair_qk = []
                for sec, jt in ((0, hp), (1, CT + hp)):  # q tile, k tile
                    colbase = sec * C + hp * P
                    pq = psqk.tile([P, N], f32, tag="pq")
                    for ct in range(CT):
                        for h in range(2):
                            nc.tensor.matmul(
                                pq[:, h * 512 : (h + 1) * 512],
                                wqk_sb[:, ct, colbase : colbase + P],
                                xT[:, ct, h * 512 : (h + 1) * 512],
                                start=(ct == 0), stop=(ct == CT - 1),
                            )
                    dst = qkT_pool.tile([P, N], f32r, tag="qkT")
                    nc.vector.tensor_scalar_add(dst, pq, bqk_sb[:, jt : jt + 1])
                    pair_qk.append(dst)
                qT_t, kT_t = pair_qk

                # --- S_T + masked exp, per head ---
                A_pair = []
                for o in (0, HD):  # head A rows 0:64, head B rows 64:128
                    A_t = A_pool.tile([P, NT, N], bf16, tag="A")
                    for tk in range(NT):
                        ps = pss.tile([P, N], f32, tag="s")
                        for h in range(2):
                            nc.tensor.matmul(
                                ps[:, h * 512 : (h + 1) * 512],
                                kT_t[o : o + HD, tk * P : (tk + 1) * P],
                                qT_t[o : o + HD, h * 512 : (h + 1) * 512],
                            )
                        # restore the always-keep diagonal: s += -8*ln(p) there
                        dg = ps[:, tk * P : (tk + 1) * P]
                        nc.vector.scalar_tensor_tensor(
                            dg, ident, n8logp[:, tk : tk + 1], dg,
                            op0=Alu.mult, op1=Alu.add,
                        )
                        # A = exp(s/8 + ln(p_key))
                        nc.scalar.activation(
                            A_t[:, tk, :], ps, Act.Exp,
                            bias=logp[:, tk : tk + 1], scale=SCALE,
                        )
                    A_pair.append(A_t)

                # --- AV + denominators (head A psum row 64, head B row 0) ---
                dsb = dsbp.tile([HD + 1, N], f32r, tag="dsb")
                for hi, (o, A_t) in enumerate(zip((0, HD), A_pair)):
                    pav = psav.tile([P, N], f32, tag="av")
                    if hi == 0:
                        vsl = (0, HD + 1)   # [vA | 1] -> rows 0:64 out, 64 denom
                        orows, drow = (0, HD), HD
                    else:
                        # window starts at head A's ones lane -> row 0 denom,
                        # vB -> rows 64:128 out
                        vsl = (HD, HD + P)
                        orows, drow = (HD, P), 0
                    mrows = vsl[1] - vsl[0]
                    for tk in range(NT):
                        lhs = vv[:, tk, hp, vsl[0] : vsl[1]]
                        for h in range(2):
                            nc.tensor.matmul(
                                pav[:mrows, h * 512 : (h + 1) * 512],
                                lhs,
                                A_t[:, tk, h * 512 : (h + 1) * 512],
                                start=(tk == 0), stop=(tk == NT - 1),
                            )
                    # denominator row stays at its psum partition (64 / 0)
                    nc.vector.tensor_copy(
                        dsb[drow : drow + 1, :], pav[drow : drow + 1, :]
                    )
                    nc.vector.tensor_copy(
                        outT[orows[0] : orows[1], hp, :], pav[orows[0] : orows[1], :]
                    )

                # --- normalize the pair: out /= (denom + eps) ---
                # PE outer-product broadcast of the two denominator rows
                pbA = psav.tile([P, N], f32, tag="av")
                pbB = psav.tile([P, N], f32, tag="av")
                for h in range(2):
                    sl = slice(h * 512, (h + 1) * 512)
                    nc.tensor.matmul(
                        pbA[0:HD, sl], onesA[HD : HD + 1, :], dsb[HD : HD + 1, sl]
                    )
                    nc.tensor.matmul(pbB[:, sl], onesB[0:1, :], dsb[0:1, sl])
                rb = rbp.tile([P, N], f32, tag="rb")
                nc.vector.tensor_scalar_add(rb[0:HD, :], pbA[0:HD, :], EPS)
                nc.vector.tensor_scalar_add(rb[HD:P, :], pbB[HD:P, :], EPS)
                nc.vector.reciprocal(rb, rb)
                nc.vector.tensor_mul(outT[:, hp, :], outT[:, hp, :], rb)

        # ---- phase 5: output projection ---------------------------------
        with tc.tile_pool(name="psy", bufs=2, space="PSUM") as psy, tc.tile_pool(
            name="yout", bufs=3
        ) as yp:
            for t in range(NT):
                py = psy.tile([P, C], f32, tag="y")
                for ct in range(CT):
                    lhs = outT[:, ct, t * P : (t + 1) * P]
                    nc.tensor.matmul(
                        py[:, 0:512], lhs, wproj_sb[:, ct, 0:512],
                        start=(ct == 0), stop=(ct == CT - 1),
                    )
                    nc.tensor.matmul(
                        py[:, 512:768], lhs, wproj_sb[:, ct, 512:768],
                        start=(ct == 0), stop=(ct == CT - 1),
                    )
                y_sb = yp.tile([P, C], f32, tag="yo")
                nc.vector.tensor_add(y_sb, py, bproj_bc)
                nc.sync.dma_start(y_v[:, t, :], y_sb)

    nc.finalize()
    _CACHE["nc"] = nc
    return nc


def make_in_maps(x, policy, W_qkv, b_qkv, W_proj, b_proj):
    x = np.asarray(x, dtype=np.float32)
    policy = np.asarray(policy, dtype=np.float32).reshape(B, N)
    shared = {
        "W_qkv": np.ascontiguousarray(np.asarray(W_qkv, dtype=np.float32)),
        "b_qkv": np.ascontiguousarray(np.asarray(b_qkv, dtype=np.float32)),
        "W_proj": np.ascontiguousarray(np.asarray(W_proj, dtype=np.float32)),
        "b_proj": np.ascontiguousarray(np.asarray(b_proj, dtype=np.float32)),
    }
    return [
        {
            "xT": np.ascontiguousarray(x[:, b, :].T),
            "policy": np.ascontiguousarray(policy[b]),
            **shared,
        }
        for b in range(B)
    ]


def kernel(x, policy, W_qkv, b_qkv, W_proj, b_proj):
    from concourse.bass_utils import run_bass_kernel_spmd

    nc = _build()
    in_maps = make_in_maps(x, policy, W_qkv, b_qkv, W_proj, b_proj)
    res = run_bass_kernel_spmd(nc, in_maps, core_ids=list(range(B)))
    y = np.stack([res.results[i]["y"] for i in range(B)], axis=1)  # (N, B, C)
    return np.ascontiguousarray(y.astype(np.float32))


# revision 44
# speedup vs baseline: 6.6393x; 6.6393x over previous
"""Policy-masked multi-head attention for Trainium2 (Bass/Tile).

Full-input contract: kernel(**inputs) takes the complete tensors and returns
the complete (N, B, C) output. Internally data-parallel over the batch dim:
core b computes batch b on one NeuronCore (no collectives). Host-side prep
is layout only: per-core slicing and an x transpose so the device streams
activations in the (c, n) orientation the PE wants.

Per-core math (N=1024 tokens, C=768, H=12 heads, hd=64):
  qkv = x @ W_qkv + b_qkv ; per head: S = q k^T / 8
  A   = exp(S) * mask      (mask = p[key] off-diag, 1 on diag)
  out = (A @ v) / (sum_k A + eps) ; y = out @ W_proj + b_proj
Softmax max-subtraction is skipped (exp args bounded, |s/8| <~ 8): identical
result modulo the ~1e-6-relative eps regularizers.

Dataflow (everything PE-friendly, no on-chip transposes):
  xT (c,n) loaded directly (float32r so the PE runs at full rate)
  qT,kT per head-pair: psum[j,n] = W_qkv[c,j]^T @ xT[c,n]; the pair's two
    heads live in the 64-row halves of the 128-row j-tile
  v in natural (n,d) orientation with an appended ones lane
  S_T[nk,nq] = kT^T qT, both heads packed in the PE array (row groups 0/64)
  policy folded into exp as a per-partition (key-indexed) log bias; the
  always-keep diagonal is restored by adding -8*ln(p) to the 128-wide
  diagonal block of the psum before the exp
  A_T[nk,nq] (bf16) -> AV with v stationary; the ones lane makes the PE
  emit softmax denominators for free (head A psum row 64, head B row 0)
  denominators broadcast across partitions with K=1 ones matmuls, then
  out_T (c,nq) is normalized in SBUF and feeds the projection as lhsT,
  producing natural (n,c) psum tiles that DMA straight to HBM.

Hardware gotchas encoded here (found empirically):
  - float32r matmul operands must come from an f32r-dtype producer (we
    declare the DRAM params f32r; DVE copies/adds produce the rest)
  - gpsimd.partition_broadcast only works from AP base partition 0 to a
    full-partition destination (so we broadcast via the PE instead)
  - two matmuls with different lhsT base partitions must not share a psum
    accumulation group (device fault)
  - engine ops need 32-aligned base partitions
"""

import sys

if "/opt/trn_rl_repo" not in sys.path:
    sys.path.insert(0, "/opt/trn_rl_repo")

import numpy as np

N, B, C = 1024, 8, 768
NH = 12          # heads
HD = 64          # head dim
P = 128          # partitions
NT = N // P      # 8 token tiles
CT = C // P      # 6 channel tiles
HP = NH // 2     # 6 head pairs
SCALE = 0.125    # hd**-0.5
EPS = 1e-6
# per (tile, pair) v-lane layout: [vA(64) | 1 | 0(63) | vB(64)] (+pad)
# head A stationary window = cols [0:65)   -> psum rows 0:64 out, row 64 denom
# head B stationary window = cols [64:192) -> psum rows 64:128 out; window
#   col 0 is head A's ones lane, so psum row 0 = head B's denominator
VW = 192

_CACHE = {}


def _build(reps=1):
    if reps in _CACHE:
        return _CACHE[reps]

    from contextlib import ExitStack

    import concourse.bass as bass
    import concourse.tile as tile
    from concourse import bacc, mybir
    from concourse.masks import make_identity

    f32 = mybir.dt.float32
    f32r = mybir.dt.float32r
    bf16 = mybir.dt.bfloat16
    Alu = mybir.AluOpType
    Act = mybir.ActivationFunctionType

    nc = bacc.Bacc()

    xT_d = nc.declare_dram_parameter("xT", [C, N], f32r, isOutput=False)
    pol_d = nc.declare_dram_parameter("policy", [N], f32, isOutput=False)
    wqkv_d = nc.declare_dram_parameter("W_qkv", [C, 3 * C], f32r, isOutput=False)
    bqkv_d = nc.declare_dram_parameter("b_qkv", [3 * C], f32, isOutput=False)
    wproj_d = nc.declare_dram_parameter("W_proj", [C, C], f32r, isOutput=False)
    bproj_d = nc.declare_dram_parameter("b_proj", [C], f32, isOutput=False)
    y_d = nc.declare_dram_parameter("y", [N, C], f32, isOutput=True)

    xT_v = xT_d.rearrange("(ct p) n -> p ct n", p=P)      # (128, 6, 1024)
    pol_v = pol_d.rearrange("(t p) -> p t", p=P)          # (128, 8)
    wqkv_v = wqkv_d.rearrange("(ct p) j -> p ct j", p=P)  # (128, 6, 2304)
    wproj_v = wproj_d.rearrange("(ct p) j -> p ct j", p=P)
    bqk_v = bqkv_d[0 : 2 * C].rearrange("(t p) -> p t", p=P)  # (128, 12)
    y_v = y_d.rearrange("(t p) c -> p t c", p=P)

    def bcast(dram_ap, parts):
        # partition-broadcast read of a 1-D dram slice (step-0 partition dim)
        return bass.AP(
            tensor=dram_ap.tensor, offset=dram_ap.offset, ap=[[0, parts]] + dram_ap.ap
        )

    with tile.TileContext(nc) as tc, ExitStack() as ctx:
        # ---- pools -------------------------------------------------------
        persist = ctx.enter_context(tc.tile_pool(name="persist", bufs=1))
        qkT_pool = ctx.enter_context(tc.tile_pool(name="qkT", bufs=3))
        A_pool = ctx.enter_context(tc.tile_pool(name="Apool", bufs=2))

        # ---- constants / weights (loaded once, reused across reps) ------
        ident = persist.tile([P, P], f32)  # diagonal mask for the keep-fix
        make_identity(nc, ident)

        pol_sb = persist.tile([P, NT], f32)
        nc.sync.dma_start(pol_sb, pol_v)
        logp = persist.tile([P, NT], f32)
        # clamp away exact zeros so ln() stays finite (ln(1e-38) = -87.5)
        nc.vector.tensor_scalar_max(logp, pol_sb, 1e-38)
        nc.scalar.activation(logp, logp, Act.Ln)
        n8logp = persist.tile([P, NT], f32)
        nc.vector.tensor_scalar_mul(n8logp, logp, -8.0)

        bqk_sb = persist.tile([P, 2 * CT], f32)
        nc.sync.dma_start(bqk_sb, bqk_v)
        bv_bc = persist.tile([P, C], f32)
        nc.sync.dma_start(bv_bc, bcast(bqkv_d[2 * C : 3 * C], P))
        bproj_bc = persist.tile([P, C], f32)
        nc.sync.dma_start(bproj_bc, bcast(bproj_d[:], P))

        onesA_f = persist.tile([HD + 1, HD], f32)
        nc.vector.memset(onesA_f, 1.0)
        onesA = persist.tile([HD + 1, HD], f32r)
        nc.vector.tensor_copy(onesA, onesA_f)
        onesB_f = persist.tile([1, P], f32)
        nc.vector.memset(onesB_f[:, 0:HD], 0.0)
        nc.vector.memset(onesB_f[:, HD:P], 1.0)
        onesB = persist.tile([1, P], f32r)
        nc.vector.tensor_copy(onesB, onesB_f)

        def emit_body(rep):
            body = ExitStack()
            bpool = body.enter_context(
                tc.tile_pool(name=f"body{rep}", bufs=1)
            )
            wvstack = ExitStack()
            wvp = wvstack.enter_context(tc.tile_pool(name=f"wv{rep}", bufs=1))

            # ---- input loads, in consumption order ----------------------
            xT = bpool.tile([P, CT, N], f32r, name="xT")       # 24 KB/part
            wv_sb = wvp.tile([P, CT, C], f32r, name="wv")      # 18 KB/part
            for ct in range(CT):
                nc.sync.dma_start(xT[:, ct, :], xT_v[:, ct, :])
                nc.sync.dma_start(wv_sb[:, ct, :], wqkv_v[:, ct, 2 * C : 3 * C])
            wqk_sb = bpool.tile([P, CT, 2 * C], f32r, name="wqk")  # 36 KB/part
            for ct in range(CT):
                nc.sync.dma_start(wqk_sb[:, ct, :], wqkv_v[:, ct, 0 : 2 * C])
            wproj_sb = bpool.tile([P, CT, C], f32r, name="wproj")  # 18 KB/part
            for ct in range(CT):
                nc.sync.dma_start(wproj_sb[:, ct, :], wproj_v[:, ct, :])

            vv = bpool.tile([P, NT, HP, VW], bf16, name="vv")  # 18 KB/part
            outT = bpool.tile([P, HP, N], f32r, name="outT")   # 24 KB/part

            # ones/zeros lanes of the v layout
            nc.gpsimd.memset(vv[:, :, :, HD : 2 * HD], 0.0)
            nc.gpsimd.memset(vv[:, :, :, HD : HD + 1], 1.0)

            # ---- phase 2: v projection (natural orientation) ------------
            with tc.tile_pool(name=f"psv{rep}", bufs=2, space="PSUM") as psv:
                for t in range(NT):
                    pv = psv.tile([P, C], f32, tag="pv")
                    for ct in range(CT):
                        lhs = xT[:, ct, t * P : (t + 1) * P]
                        nc.tensor.matmul(
                            pv[:, 0:512], lhs, wv_sb[:, ct, 0:512],
                            start=(ct == 0), stop=(ct == CT - 1),
                        )
                        nc.tensor.matmul(
                            pv[:, 512:768], lhs, wv_sb[:, ct, 512:768],
                            start=(ct == 0), stop=(ct == CT - 1),
                        )
                    # scatter halves into the v layout, add bias, cast bf16
                    pv3 = pv.rearrange("p (hp w) -> p hp w", w=P)
                    bv3 = bv_bc.rearrange("p (hp w) -> p hp w", w=P)
                    nc.vector.tensor_add(
                        vv[:, t, :, 0:HD], pv3[:, :, 0:HD], bv3[:, :, 0:HD]
                    )
                    nc.vector.tensor_add(
                        vv[:, t, :, P : P + HD], pv3[:, :, HD:P], bv3[:, :, HD:P]
                    )
            wvstack.close()  # wv consumed; free its SBUF

            # ---- phase 3: per head-pair: qT,kT -> S_T -> exp -> AV ------
            with tc.tile_pool(
                name=f"psqk{rep}", bufs=1, space="PSUM"
            ) as psqk, tc.tile_pool(
                name=f"pss{rep}", bufs=2, space="PSUM"
            ) as pss, tc.tile_pool(
                name=f"psav{rep}", bufs=1, space="PSUM"
            ) as psav, tc.tile_pool(
                name=f"dsbp{rep}", bufs=2
            ) as dsbp, tc.tile_pool(name=f"rbp{rep}", bufs=2) as rbp:
                for hp in range(HP):
                    # --- qT, kT for this pair (two 128-row j-tiles) ---
                    pair_qk = []
                    for sec, jt in ((0, hp), (1, CT + hp)):  # q tile, k tile
                        colbase = sec * C + hp * P
                        pq = psqk.tile([P, N], f32, tag="pq")
                        for ct in range(CT):
                            for h in range(2):
                                nc.tensor.matmul(
                                    pq[:, h * 512 : (h + 1) * 512],
                                    wqk_sb[:, ct, colbase : colbase + P],
                                    xT[:, ct, h * 512 : (h + 1) * 512],
                                    start=(ct == 0), stop=(ct == CT - 1),
                                )
                        dst = qkT_pool.tile([P, N], f32r, tag="qkT")
                        nc.vector.tensor_scalar_add(dst, pq, bqk_sb[:, jt : jt + 1])
                        pair_qk.append(dst)
                    qT_t, kT_t = pair_qk

                    # --- S_T + masked exp, per head ---
                    A_pair = []
                    for o in (0, HD):  # head A rows 0:64, head B rows 64:128
                        A_t = A_pool.tile([P, NT, N], bf16, tag="A")
                        for tk in range(NT):
                            ps = pss.tile([P, N], f32, tag="s")
                            for h in range(2):
                                nc.tensor.matmul(
                                    ps[:, h * 512 : (h + 1) * 512],
                                    kT_t[o : o + HD, tk * P : (tk + 1) * P],
                                    qT_t[o : o + HD, h * 512 : (h + 1) * 512],
                                )
                            # restore always-keep diagonal: s += -8*ln(p)
                            dg = ps[:, tk * P : (tk + 1) * P]
                            nc.vector.scalar_tensor_tensor(
                                dg, ident, n8logp[:, tk : tk + 1], dg,
                                op0=Alu.mult, op1=Alu.add,
                            )
                            # A = exp(s/8 + ln(p_key))
                            nc.scalar.activation(
                                A_t[:, tk, :], ps, Act.Exp,
                                bias=logp[:, tk : tk + 1], scale=SCALE,
                            )
                        A_pair.append(A_t)

                    # --- AV + denominators (head A row 64, head B row 0) ---
                    dsb = dsbp.tile([HD + 1, N], f32r, tag="dsb")
                    for hi, A_t in enumerate(A_pair):
                        pav = psav.tile([P, N], f32, tag="av")
                        if hi == 0:
                            vsl = (0, HD + 1)   # [vA | 1] -> 0:64 out, 64 denom
                            orows, drow = (0, HD), HD
                        else:
                            vsl = (HD, HD + P)  # -> 64:128 out, 0 denom
                            orows, drow = (HD, P), 0
                        mrows = vsl[1] - vsl[0]
                        for tk in range(NT):
                            lhs = vv[:, tk, hp, vsl[0] : vsl[1]]
                            for h in range(2):
                                nc.tensor.matmul(
                                    pav[:mrows, h * 512 : (h + 1) * 512],
                                    lhs,
                                    A_t[:, tk, h * 512 : (h + 1) * 512],
                                    start=(tk == 0), stop=(tk == NT - 1),
                                )
                        nc.vector.tensor_copy(
                            dsb[drow : drow + 1, :], pav[drow : drow + 1, :]
                        )
                        nc.vector.tensor_copy(
                            outT[orows[0] : orows[1], hp, :],
                            pav[orows[0] : orows[1], :],
                        )

                    # --- normalize the pair: out /= (denom + eps) ---
                    # PE outer-product broadcast of the two denominator rows
                    pbA = psav.tile([P, N], f32, tag="av")
                    pbB = psav.tile([P, N], f32, tag="av")
                    for h in range(2):
                        sl = slice(h * 512, (h + 1) * 512)
                        nc.tensor.matmul(
                            pbA[0:HD, sl], onesA[HD : HD + 1, :], dsb[HD : HD + 1, sl]
                        )
                        nc.tensor.matmul(pbB[:, sl], onesB[0:1, :], dsb[0:1, sl])
                    rb = rbp.tile([P, N], f32, tag="rb")
                    nc.vector.tensor_scalar_add(rb[0:HD, :], pbA[0:HD, :], EPS)
                    nc.vector.tensor_scalar_add(rb[HD:P, :], pbB[HD:P, :], EPS)
                    nc.vector.reciprocal(rb, rb)
                    nc.vector.tensor_mul(outT[:, hp, :], outT[:, hp, :], rb)

            # ---- phase 5: output projection ------------------------------
            with tc.tile_pool(
                name=f"psy{rep}", bufs=2, space="PSUM"
            ) as psy, tc.tile_pool(name=f"yout{rep}", bufs=3) as yp:
                for t in range(NT):
                    py = psy.tile([P, C], f32, tag="y")
                    for ct in range(CT):
                        lhs = outT[:, ct, t * P : (t + 1) * P]
                        nc.tensor.matmul(
                            py[:, 0:512], lhs, wproj_sb[:, ct, 0:512],
                            start=(ct == 0), stop=(ct == CT - 1),
                        )
                        nc.tensor.matmul(
                            py[:, 512:768], lhs, wproj_sb[:, ct, 512:768],
                            start=(ct == 0), stop=(ct == CT - 1),
                        )
                    y_sb = yp.tile([P, C], f32, tag="yo")
                    nc.vector.tensor_add(y_sb, py, bproj_bc)
                    nc.sync.dma_start(y_v[:, t, :], y_sb)
            body.close()

        for rep in range(reps):
            emit_body(rep)

    nc.finalize()
    _CACHE[reps] = nc
    return nc


def make_in_maps(x, policy, W_qkv, b_qkv, W_proj, b_proj):
    x = np.asarray(x, dtype=np.float32)
    policy = np.asarray(policy, dtype=np.float32).reshape(B, N)
    shared = {
        "W_qkv": np.ascontiguousarray(np.asarray(W_qkv, dtype=np.float32)),
        "b_qkv": np.ascontiguousarray(np.asarray(b_qkv, dtype=np.float32)),
        "W_proj": np.ascontiguousarray(np.asarray(W_proj, dtype=np.float32)),
        "b_proj": np.ascontiguousarray(np.asarray(b_proj, dtype=np.float32)),
    }
    return [
        {
            "xT": np.ascontiguousarray(x[:, b, :].T),
            "policy": np.ascontiguousarray(policy[b]),
            **shared,
        }
        for b in range(B)
    ]


def kernel(x, policy, W_qkv, b_qkv, W_proj, b_proj):
    from concourse.bass_utils import run_bass_kernel_spmd

    nc = _build()
    in_maps = make_in_maps(x, policy, W_qkv, b_qkv, W_proj, b_proj)
    res = run_bass_kernel_spmd(nc, in_maps, core_ids=list(range(B)))
    y = np.stack([res.results[i]["y"] for i in range(B)], axis=1)  # (N, B, C)
    return np.ascontiguousarray(y.astype(np.float32))
